# revision 4
# baseline (speedup 1.0000x reference)
"""nn_KNN Trainium2 kernel: sequential per-node neighbor-mean scan as one GEMM.

The reference's scan is a composition of per-column linear updates, so
out = x0 @ M for a precomputable M. Folding the initial mask-fill into M
(zeroing the unknown rows -> M', bias r), known columns pass through
exactly and only the 256 unknown columns need compute:

  out[:, known]   = input[:, known]            (host-side pass-through)
  out[:, unknown] = input[:, known] @ Vk + r,  Vk = M'[known][:, unknown]

Sharding: batch b -> core b (data parallel, no collectives). Each core
gets its shard's known rows pre-tiled in fp8e4 as 8 time-slices
xP [8, 128, 6*512] (partition-contiguous 3KB DMA lines) plus a
pre-swizzled Vk (vkP, contiguous 1536B lines). It computes
outU [256, 4096] = Vk.T @ xT with DoubleRow fp8 matmuls and writes outU
in fp8; the host adds the bias r and restores f32.

Schedule (from perfetto iteration): ~16.5us of the span is fixed NEFF
preamble/postamble, so the body is tuned to stream flat-out:
 - x slices ride the two HWDGE rings (qSP starts ~1us before qAct, so
   vk + the early slices lean on sync); slices 0/1/7 are split across
   both rings so the MM stream never starves.
 - A fine-grained N=128 dummy-matmul stream bridges the PE from the
   preamble barrier to the first real matmul, keeping the HAM clock
   warm (2.4GHz) for the whole real MM stream.
 - Each slice accumulates both 128-row output blocks into one 2-bank
   PSUM tile; a single wide PSUM->SBUF fp8 cast per slice (vector,
   with scalar helping on 2 slices) keeps the DMA-issuing engines free.
 - Stores are fp8 pairs of slices pushed onto the HWDGE rings behind
   the loads (FIFO keeps the rings saturated; each push reuses a DMA
   sem lane whose previous consumer finished long before).
"""

import sys

import numpy as np

try:
    import concourse.bass  # noqa: F401
except ImportError:  # pragma: no cover
    sys.path.insert(0, "/opt/trn_rl_repo")

import ml_dtypes

import concourse.bacc as bacc_mod
import concourse.mybir as mybir
from concourse.bass_utils import run_bass_kernel_spmd
from concourse.tile import TileContext

B, T, N, NS = 8, 4096, 1024, 256
NK = N - NS
P = 128
TW = 512
NL = T // TW
JC = NK // P        # 6 contraction chunks of 128
CP = JC // 2        # 3 DoubleRow chunk-pairs
SB = NS // P        # 2 output partition blocks

FP8 = ml_dtypes.float8_e4m3


def _build_kernel(warmups=30, ps_bufs=3, ot_bufs=3):
    nc = bacc_mod.Bacc("TRN2", target_bir_lowering=False, name="knn_fp8")
    f32 = mybir.dt.float32
    fp8 = mybir.dt.float8e4
    xP = nc.dram_tensor("xP", [NL, P, JC * TW], fp8, kind="ExternalInput")
    vkP = nc.dram_tensor("vkP", [P, JC * NS], fp8, kind="ExternalInput")
    outU = nc.dram_tensor("outU", [NS, T], fp8, kind="ExternalOutput")
    outUr = outU.rearrange("(s p) t -> p s t", p=P)  # [128, 2, T]

    with TileContext(nc) as tc:
        with (
            tc.tile_pool(name="consts", bufs=1) as cpool,
            tc.tile_pool(name="xt", bufs=NL) as xpool,
            tc.tile_pool(name="outp", bufs=ot_bufs) as opool,
            tc.tile_pool(name="wps", bufs=1, space="PSUM") as wpool,
            tc.tile_pool(name="ps", bufs=ps_bufs, space="PSUM") as pspool,
        ):
            vk_sb = cpool.tile([P, JC * NS], fp8, tag="vk")
            nc.sync.dma_start(out=vk_sb, in_=vkP[:, :])
            vk3 = vk_sb.rearrange("p (c s) -> p c s", c=JC)

            half = JC * TW // 2
            xts = []
            xtiles = []
            for t in range(NL):
                xt = xpool.tile([P, JC * TW], fp8, tag="xt", name=f"xt{t}")
                xtiles.append(xt)
                xts.append(xt.rearrange("p (c f) -> p c f", c=JC))
            # early slices lean on the early-starting sync ring; x0/x1/x7
            # split across both rings so arrivals track compute order
            nc.scalar.dma_start(out=xtiles[0][:, :half], in_=xP[0][:, :half])
            nc.sync.dma_start(out=xtiles[0][:, half:], in_=xP[0][:, half:])
            nc.scalar.dma_start(out=xtiles[1][:, :half], in_=xP[1][:, :half])
            nc.sync.dma_start(out=xtiles[1][:, half:], in_=xP[1][:, half:])
            nc.scalar.dma_start(out=xtiles[2], in_=xP[2])
            nc.sync.dma_start(out=xtiles[3], in_=xP[3])
            nc.sync.dma_start(out=xtiles[4], in_=xP[4])
            nc.scalar.dma_start(out=xtiles[5], in_=xP[5])
            nc.scalar.dma_start(out=xtiles[6], in_=xP[6])
            nc.scalar.dma_start(out=xtiles[7][:, :half], in_=xP[7][:, :half])
            nc.sync.dma_start(out=xtiles[7][:, half:], in_=xP[7][:, half:])

            # dummy-matmul bridge keeps the PE HAM-warm until slice 0
            # lands; scalar.add pulls ACT_TABLE_LOAD off the critical path
            scr = cpool.tile([P, 128], fp8, tag="scr")
            nc.vector.memset(scr, 0)
            scr2 = cpool.tile([P, 1], f32, tag="scr2")
            nc.scalar.add(scr2, scr[:, :1], 0.0)
            wps = wpool.tile([P, 128], f32, tag="wps")
            for w in range(warmups):
                nc.tensor.matmul(wps, lhsT=scr, rhs=scr, start=True,
                                 stop=True)

            ot2 = None
            for t in range(NL):
                if t % 2 == 0:
                    ot2 = opool.tile([P, SB * 2 * TW], fp8, tag="ot",
                                     name=f"ot{t}")
                    o4 = ot2.rearrange("p (s u w) -> p s u w", s=SB, u=2)
                u = t % 2
                ps = pspool.tile([P, SB * TW], f32, tag="ps", name=f"ps{t}")
                for sb in range(SB):
                    for cp in range(CP):
                        nc.tensor.matmul(
                            ps[:, sb * TW:(sb + 1) * TW],
                            lhsT=vk3[:, 2 * cp:2 * cp + 2,
                                     sb * P:(sb + 1) * P],
                            rhs=xts[t][:, 2 * cp:2 * cp + 2, :],
                            start=(cp == 0),
                            stop=(cp == CP - 1),
                            perf_mode=mybir.MatmulPerfMode.DoubleRow,
                        )
                ps2 = ps.rearrange("p (s w) -> p s w", s=SB)
                if t == NL - 1:
                    # final slice: copies run in parallel for latency
                    nc.vector.tensor_copy(o4[:, 0, u, :], ps2[:, 0, :])
                    nc.scalar.add(o4[:, 1, u, :], ps2[:, 1, :], 0.0)
                else:
                    if t in (2, 5):
                        nc.scalar.copy(o4[:, :, u, :], ps2)
                    else:
                        nc.vector.tensor_copy(o4[:, :, u, :], ps2)

                if t % 2 == 1 and t < NL - 2:
                    dst = outUr[:, :, (t - 1) * TW:(t + 1) * TW]
                    src = ot2.rearrange("p (s w) -> p s w", s=SB)
                    eng = nc.sync if t == 1 else nc.scalar
                    eng.dma_start(out=dst, in_=src)
                elif t == NL - 2:
                    dst = outUr[:, :, t * TW:(t + 1) * TW]
                    nc.sync.dma_start(out=dst, in_=o4[:, :, 0, :])
                elif t == NL - 1:
                    dst = outUr[:, :, t * TW:(t + 1) * TW]
                    nc.sync.dma_start(out=dst[:, 0, :], in_=o4[:, 0, 1, :])
                    nc.scalar.dma_start(out=dst[:, 1, :],
                                        in_=o4[:, 1, 1, :])
    nc.compile()
    return nc


_NC_CACHE = {}


def _get_nc():
    if "nc" not in _NC_CACHE:
        _NC_CACHE["nc"] = _build_kernel()
    return _NC_CACHE["nc"]


def _derive_operator(A, unknown, mask):
    """Compose the scan into (Vk, rS, known) in float64."""
    A64 = np.asarray(A, dtype=np.float64)
    deg = A64.sum(axis=1)
    M = np.eye(N, dtype=np.float64)
    for u in unknown:
        M[:, u] = M @ (A64[u] / deg[u])
    r = float(mask) * M[unknown, :].sum(axis=0)
    M[unknown, :] = 0.0
    known = np.setdiff1d(np.arange(N, dtype=np.int64), unknown)
    Vk = M[known][:, unknown]
    rS = np.ascontiguousarray(r[unknown], dtype=np.float32)
    return Vk, rS, known


def _prep_in_maps(x, Vk, rS, known):
    # vkP[p, c*NS+s] = Vk[c*128+p, s]  (contiguous per-partition lines)
    vkP = np.ascontiguousarray(
        Vk.astype(FP8).reshape(JC, P, NS).transpose(1, 0, 2)
    ).reshape(P, JC * NS)
    in_maps = []
    for b in range(B):
        xT = np.ascontiguousarray(x[b].T[known]).astype(FP8)  # [768, 4096]
        # slice-major tiling: xP[l, p, c*TW + u] = xT[c*128 + p, l*TW + u]
        xPa = np.ascontiguousarray(
            xT.reshape(JC, P, NL, TW).transpose(2, 1, 0, 3)
        ).reshape(NL, P, JC * TW)
        in_maps.append({"xP": xPa, "vkP": vkP})
    return in_maps


def kernel(input, A, unknown, mask, _spmd_kwargs=None):
    x = np.asarray(input, dtype=np.float32)
    unknown = np.asarray(unknown).astype(np.int64)
    Vk, rS, known = _derive_operator(A, unknown, mask)
    in_maps = _prep_in_maps(x, Vk, rS, known)

    nc = _get_nc()
    res = run_bass_kernel_spmd(nc, in_maps, core_ids=list(range(B)),
                               **(_spmd_kwargs or {}))

    out = x.copy()
    for b in range(B):
        out[b][:, unknown] = (
            res.results[b]["outU"].T.astype(np.float32) + rS[None, :]
        )
    return out


# revision 6
# speedup vs baseline: 1.0709x; 1.0709x over previous
"""nn_KNN Trainium2 kernel: sequential per-node neighbor-mean scan as one GEMM.

The reference's scan is a composition of per-column linear updates, so
out = x0 @ M for a precomputable M. Folding the initial mask-fill into M
(zeroing the unknown rows -> M', bias r), known columns pass through
exactly and only the 256 unknown columns need compute:

  out[:, known]   = input[:, known]            (host-side pass-through)
  out[:, unknown] = input[:, known] @ Vk + r,  Vk = M'[known][:, unknown]

Sharding: batch b -> core b (data parallel, no collectives). Each core
gets its shard's known rows pre-tiled in fp8e4 as 8 time-slices
xP [8, 128, 6*512] (partition-contiguous 3KB DMA lines) plus a
pre-swizzled Vk (vkP, contiguous 1536B lines). It computes
outU [256, 4096] = Vk.T @ xT with DoubleRow fp8 matmuls and writes outU
in fp8; the host adds the bias r and restores f32.

Schedule (from perfetto iteration): ~16.5us of the span is fixed NEFF
preamble/postamble, so the body is tuned to stream flat-out:
 - x slices ride the two HWDGE rings (qSP starts ~1us before qAct, so
   vk + the early slices lean on sync); slices 0/1/7 are split across
   both rings so the MM stream never starves.
 - A fine-grained N=128 dummy-matmul stream bridges the PE from the
   preamble barrier to the first real matmul, keeping the HAM clock
   warm (2.4GHz) for the whole real MM stream.
 - Each slice accumulates both 128-row output blocks into one 2-bank
   PSUM tile; a single wide PSUM->SBUF fp8 cast per slice (vector,
   with scalar helping on 2 slices) keeps the DMA-issuing engines free.
 - Stores are fp8 pairs of slices pushed onto the HWDGE rings behind
   the loads (FIFO keeps the rings saturated; each push reuses a DMA
   sem lane whose previous consumer finished long before).
"""

import sys

import numpy as np

try:
    import concourse.bass  # noqa: F401
except ImportError:  # pragma: no cover
    sys.path.insert(0, "/opt/trn_rl_repo")

import ml_dtypes

import concourse.bacc as bacc_mod
import concourse.mybir as mybir
from concourse.bass_utils import run_bass_kernel_spmd
from concourse.tile import TileContext

B, T, N, NS = 8, 4096, 1024, 256
NK = N - NS
P = 128
TW = 512
NL = T // TW
JC = NK // P        # 6 contraction chunks of 128
CP = JC // 2        # 3 DoubleRow chunk-pairs
SB = NS // P        # 2 output partition blocks

FP8 = ml_dtypes.float8_e4m3


def _build_kernel(warmups=24, prelude=18, ps_bufs=3, ot_bufs=3):
    nc = bacc_mod.Bacc("TRN2", target_bir_lowering=False, name="knn_fp8")
    f32 = mybir.dt.float32
    fp8 = mybir.dt.float8e4
    xP = nc.dram_tensor("xP", [NL, P, JC * TW], fp8, kind="ExternalInput")
    vkP = nc.dram_tensor("vkP", [P, JC * NS], fp8, kind="ExternalInput")
    outU = nc.dram_tensor("outU", [NS, T], fp8, kind="ExternalOutput")
    outUr = outU.rearrange("(s p) t -> p s t", p=P)  # [128, 2, T]

    with TileContext(nc) as tc:
        with (
            tc.tile_pool(name="consts", bufs=1) as cpool,
            tc.tile_pool(name="xt", bufs=NL) as xpool,
            tc.tile_pool(name="outp", bufs=ot_bufs) as opool,
            tc.tile_pool(name="wps", bufs=1, space="PSUM") as wpool,
            tc.tile_pool(name="ps", bufs=ps_bufs, space="PSUM") as pspool,
        ):
            vk_sb = cpool.tile([P, JC * NS], fp8, tag="vk")
            nc.sync.dma_start(out=vk_sb, in_=vkP[:, :])
            vk3 = vk_sb.rearrange("p (c s) -> p c s", c=JC)

            half = JC * TW // 2
            xts = []
            xtiles = []
            for t in range(NL):
                xt = xpool.tile([P, JC * TW], fp8, tag="xt", name=f"xt{t}")
                xtiles.append(xt)
                xts.append(xt.rearrange("p (c f) -> p c f", c=JC))
            # early slices lean on the early-starting sync ring; x0/x1/x7
            # split across both rings so arrivals track compute order
            nc.scalar.dma_start(out=xtiles[0][:, :half], in_=xP[0][:, :half])
            nc.sync.dma_start(out=xtiles[0][:, half:], in_=xP[0][:, half:])
            nc.scalar.dma_start(out=xtiles[1][:, :half], in_=xP[1][:, :half])
            nc.sync.dma_start(out=xtiles[1][:, half:], in_=xP[1][:, half:])
            nc.scalar.dma_start(out=xtiles[2], in_=xP[2])
            nc.sync.dma_start(out=xtiles[3], in_=xP[3])
            nc.sync.dma_start(out=xtiles[4], in_=xP[4])
            nc.scalar.dma_start(out=xtiles[5], in_=xP[5])
            nc.scalar.dma_start(out=xtiles[6], in_=xP[6])
            nc.scalar.dma_start(out=xtiles[7][:, :half], in_=xP[7][:, :half])
            nc.sync.dma_start(out=xtiles[7][:, half:], in_=xP[7][:, half:])

            # dummy-matmul bridge keeps the PE HAM-warm until slice 0
            # lands; the zero-dependency const-AP prelude starts the PE
            # the instant it exits the entry barrier (~0.6us earlier),
            # then the scr stream takes over once the memset lands.
            # scalar.add pulls ACT_TABLE_LOAD off the critical path.
            wps = wpool.tile([P, 128], f32, tag="wps")
            c0 = nc.const_aps.aps[(f32, 0.0)]
            for w in range(prelude):
                nc.tensor.matmul(wps[:1, :1], lhsT=c0, rhs=c0, start=True,
                                 stop=True)
            scr = cpool.tile([P, 128], fp8, tag="scr")
            nc.vector.memset(scr, 0)
            scr2 = cpool.tile([P, 1], f32, tag="scr2")
            nc.scalar.add(scr2, scr[:, :1], 0.0)
            for w in range(warmups):
                nc.tensor.matmul(wps, lhsT=scr, rhs=scr, start=True,
                                 stop=True)

            ot2 = None
            for t in range(NL):
                if t % 2 == 0:
                    ot2 = opool.tile([P, SB * 2 * TW], fp8, tag="ot",
                                     name=f"ot{t}")
                    o4 = ot2.rearrange("p (s u w) -> p s u w", s=SB, u=2)
                u = t % 2
                ps = pspool.tile([P, SB * TW], f32, tag="ps", name=f"ps{t}")
                for sb in range(SB):
                    for cp in range(CP):
                        nc.tensor.matmul(
                            ps[:, sb * TW:(sb + 1) * TW],
                            lhsT=vk3[:, 2 * cp:2 * cp + 2,
                                     sb * P:(sb + 1) * P],
                            rhs=xts[t][:, 2 * cp:2 * cp + 2, :],
                            start=(cp == 0),
                            stop=(cp == CP - 1),
                            perf_mode=mybir.MatmulPerfMode.DoubleRow,
                        )
                ps2 = ps.rearrange("p (s w) -> p s w", s=SB)
                if t == NL - 1:
                    # final slice: copies run in parallel for latency
                    nc.vector.tensor_copy(o4[:, 0, u, :], ps2[:, 0, :])
                    nc.scalar.add(o4[:, 1, u, :], ps2[:, 1, :], 0.0)
                else:
                    if t in (2, 5):
                        nc.scalar.copy(o4[:, :, u, :], ps2)
                    else:
                        nc.vector.tensor_copy(o4[:, :, u, :], ps2)

                if t % 2 == 1 and t < NL - 2:
                    dst = outUr[:, :, (t - 1) * TW:(t + 1) * TW]
                    src = ot2.rearrange("p (s w) -> p s w", s=SB)
                    eng = nc.sync if t == 1 else nc.scalar
                    eng.dma_start(out=dst, in_=src)
                elif t == NL - 2:
                    dst = outUr[:, :, t * TW:(t + 1) * TW]
                    nc.sync.dma_start(out=dst, in_=o4[:, :, 0, :])
                elif t == NL - 1:
                    dst = outUr[:, :, t * TW:(t + 1) * TW]
                    nc.sync.dma_start(out=dst[:, 0, :], in_=o4[:, 0, 1, :])
                    nc.scalar.dma_start(out=dst[:, 1, :],
                                        in_=o4[:, 1, 1, :])
    nc.compile()
    return nc


_NC_CACHE = {}


def _get_nc():
    if "nc" not in _NC_CACHE:
        _NC_CACHE["nc"] = _build_kernel()
    return _NC_CACHE["nc"]


def _derive_operator(A, unknown, mask):
    """Compose the scan into (Vk, rS, known) in float64."""
    A64 = np.asarray(A, dtype=np.float64)
    deg = A64.sum(axis=1)
    M = np.eye(N, dtype=np.float64)
    for u in unknown:
        M[:, u] = M @ (A64[u] / deg[u])
    r = float(mask) * M[unknown, :].sum(axis=0)
    M[unknown, :] = 0.0
    known = np.setdiff1d(np.arange(N, dtype=np.int64), unknown)
    Vk = M[known][:, unknown]
    rS = np.ascontiguousarray(r[unknown], dtype=np.float32)
    return Vk, rS, known


def _prep_in_maps(x, Vk, rS, known):
    # vkP[p, c*NS+s] = Vk[c*128+p, s]  (contiguous per-partition lines)
    vkP = np.ascontiguousarray(
        Vk.astype(FP8).reshape(JC, P, NS).transpose(1, 0, 2)
    ).reshape(P, JC * NS)
    in_maps = []
    for b in range(B):
        xT = np.ascontiguousarray(x[b].T[known]).astype(FP8)  # [768, 4096]
        # slice-major tiling: xP[l, p, c*TW + u] = xT[c*128 + p, l*TW + u]
        xPa = np.ascontiguousarray(
            xT.reshape(JC, P, NL, TW).transpose(2, 1, 0, 3)
        ).reshape(NL, P, JC * TW)
        in_maps.append({"xP": xPa, "vkP": vkP})
    return in_maps


def kernel(input, A, unknown, mask, _spmd_kwargs=None):
    x = np.asarray(input, dtype=np.float32)
    unknown = np.asarray(unknown).astype(np.int64)
    Vk, rS, known = _derive_operator(A, unknown, mask)
    in_maps = _prep_in_maps(x, Vk, rS, known)

    nc = _get_nc()
    res = run_bass_kernel_spmd(nc, in_maps, core_ids=list(range(B)),
                               **(_spmd_kwargs or {}))

    out = x.copy()
    for b in range(B):
        out[b][:, unknown] = (
            res.results[b]["outU"].T.astype(np.float32) + rS[None, :]
        )
    return out
